# revision 1
# baseline (speedup 1.0000x reference)
"""Trainium2 Bass kernel for nn_MatchSegmentation.

Computes matching = argmin_g BCE(segmentation_k, gt_g) for K=128 proposals vs
G=gt_plane_num ground-truth masks over N=65536 pixels, sharded over the pixel
dimension across 8 NeuronCores.

Math: ce[k,g] = -(A[k,g] + B[k] - C[k,g]) / n with
  A = log(s+eps) @ g^T,  C = log(1-s+eps) @ g^T,  B = rowsum(log(1-s+eps)).
B is a per-row constant and -1/n a negative scale, so
  argmin_g ce[k,:] == argmin_g (C - A)[k,:].
Per 128-pixel chunk (contraction on the partition axis):
  ACT:  log(s+eps) and log(1-s+eps)  -> bf16, concatenated (128, 256)
  PE:   psAC(g, :) += gt_chunk^T @ [log_s | log_1ms]  (bf16 matmul, fp32 PSUM)
Each core emits its partial (GP,K) C-A over its pixel shard; the gather step
sums the 8 tiny partials, masks padded instance slots, and takes the argmin.

MODE="device" keeps a fully on-device epilogue (AllGather + replicated
argmin via max_index on -D) for reference; it is much slower end-to-end
because every core then absorbs the multi-core launch skew at the collective.
"""

import numpy as np
import ml_dtypes
from contextlib import ExitStack

import concourse.bass as bass
import concourse.tile as tile
from concourse import bacc, mybir
from concourse.bass_utils import run_bass_kernel_spmd

F32 = mybir.dt.float32
BF16 = mybir.dt.bfloat16

NCORES = 8
N_FULL = 65536          # h*w pixels
K = 128                 # segmentation channels
GMAX = 21               # gt instances provided
GP = 22                 # padded instance slots (col 21 always padding)
NSHARD = N_FULL // NCORES   # 8192 pixels per core
CHUNK = 128             # pixels per matmul (contraction = partition dim)
NCHUNK = NSHARD // CHUNK    # 64
BLOCKS = [4, 8, 16, 20, 12, 4]   # chunks per pipeline block (tapered both ends)
assert sum(BLOCKS) == NCHUNK
EPS = 1e-6

_PROG = {}  # mode -> compiled program
MODE = "host"


def _build_program(mode):
    nc = bacc.Bacc(
        "TRN2",
        target_bir_lowering=False,
        debug=False,
        enable_asserts=False,
        num_devices=NCORES,
    )

    # seg is host-pre-swizzled so partition p holds pixels {c*128+p} contiguously:
    # seg[p, gc*K + k] = segmentation_shard[gc*128 + p, k], quantized to uint16
    # (s_q = round(s*65536); dequant is exact inside the ACT affine, and the
    # <=2^-17 quantization error is ~40x below the argmin safety margin).
    seg_d = nc.dram_tensor("seg", [128, NCHUNK * K], mybir.dt.uint16, kind="ExternalInput")
    gt_d = nc.dram_tensor("gt", [128, NCHUNK * GP], BF16, kind="ExternalInput")
    bias_d = nc.dram_tensor("bias2", [128, 2], F32, kind="ExternalInput")
    if mode == "device":
        pen_d = nc.dram_tensor("pen", [GP, 1], F32, kind="ExternalInput")
        idn_d = nc.dram_tensor("idn", [GP, GP], F32, kind="ExternalInput")
        out_d = nc.dram_tensor("out", [K, 1], mybir.dt.int32, kind="ExternalOutput")
    else:
        out_d = nc.dram_tensor("out", [GP, K], F32, kind="ExternalOutput")


    nblk = len(BLOCKS)
    with tile.TileContext(nc) as tc, ExitStack() as ctx:
        # One buffer per block everywhere: the whole shard fits in SBUF, so no
        # DMA ever waits on a slot release (slot waits serialized the
        # in-order per-engine descriptor-gen streams).
        segp = ctx.enter_context(tc.tile_pool(name="segp", bufs=1))
        logp = ctx.enter_context(tc.tile_pool(name="logp", bufs=1))
        gtp = ctx.enter_context(tc.tile_pool(name="gtp", bufs=1))
        psp = ctx.enter_context(tc.tile_pool(name="psp", bufs=1, space="PSUM"))
        sml = ctx.enter_context(tc.tile_pool(name="sml", bufs=1))
        drm = ctx.enter_context(tc.tile_pool(name="drm", bufs=1, space="DRAM"))

        # Warm the ACT Ln table immediately (otherwise the pseudo table-load
        # slides to just before the first data-gated LN and serializes).
        dummy = sml.tile([1, 8], F32)
        nc.vector.memset(dummy[:], 1.0)
        nc.scalar.activation(dummy[:], dummy[:], mybir.ActivationFunctionType.Ln)

        # Small constants (gpsimd queue: keep the sync ring clear for seg).
        bias_t = sml.tile([128, 2], F32)
        nc.gpsimd.dma_start(bias_t[:], bias_d.ap())

        if mode == "device":
            pen_t = sml.tile([GP, 1], F32)
            nc.sync.dma_start(pen_t[:], pen_d.ap())
            idn_t = sml.tile([GP, GP], F32)
            nc.sync.dma_start(idn_t[:], idn_d.ap())

        # A|C accumulator: [:, :K] accumulates g^T@log_s, [:, K:] g^T@log_1ms.
        psAC = psp.tile([GP, 2 * K], F32)

        seg_ap = seg_d.ap()
        gt_ap = gt_d.ap()

        off = 0
        for b, nch in enumerate(BLOCKS):
            seg_t = segp.tile([128, nch, K], mybir.dt.uint16, name="seg_t", tag=f"seg_t{b}")
            seg_src = seg_ap[:, off * K : (off + nch) * K].rearrange(
                "p (c k) -> p c k", c=nch
            )
            # Split each block across the HWDGE (sync) and SWDGE (gpsimd)
            # rings: both queues stream concurrently at aggregate HBM rate
            # while blocks still complete in consumption order.
            h = nch // 2
            if h:
                nc.sync.dma_start(seg_t[:, :h, :], seg_src[:, :h, :])
                nc.gpsimd.dma_start(seg_t[:, h:, :], seg_src[:, h:, :])
            else:
                nc.sync.dma_start(seg_t[:], seg_src)

            gt_t = gtp.tile([128, nch, GP], BF16, name="gt_t", tag=f"gt_t{b}")
            nc.gpsimd.dma_start(
                gt_t[:],
                gt_ap[:, off * GP : (off + nch) * GP].rearrange(
                    "p (c j) -> p c j", c=nch
                ),
            )

            logs_t = logp.tile([128, nch, 2 * K], BF16, name="logs_t", tag=f"logs_t{b}")
            # log(s + eps) with s = u * 2^-16
            nc.scalar.activation(
                logs_t[:, :, 0:K], seg_t[:],
                mybir.ActivationFunctionType.Ln,
                bias=bias_t[:, 0:1], scale=1.0 / 65536.0,
            )
            # log(1 - s + eps) = log(-u * 2^-16 + (1+eps))
            nc.scalar.activation(
                logs_t[:, :, K : 2 * K], seg_t[:],
                mybir.ActivationFunctionType.Ln,
                bias=bias_t[:, 1:2], scale=-1.0 / 65536.0,
            )

            for c in range(nch):
                gc = off + c
                nc.tensor.matmul(
                    psAC[:],
                    lhsT=gt_t[:, c, :],
                    rhs=logs_t[:, c, :],
                    start=(gc == 0),
                    stop=(gc == NCHUNK - 1),
                )
            off += nch

        # D_local = C - A  (GP, K); gather step argmins the summed partials.
        ac_sb = sml.tile([GP, 2 * K], F32)
        nc.vector.tensor_copy(ac_sb[:], psAC[:])
        dt_sb = sml.tile([GP, K], F32)
        nc.vector.tensor_sub(dt_sb[:], ac_sb[:, K : 2 * K], ac_sb[:, 0:K])

        if mode == "host":
            nc.sync.dma_start(out_d.ap(), dt_sb[:])
        else:
            # AllGather partials across the 8 cores, then reduce locally.
            cc_in = drm.tile([GP, K], F32)
            nc.sync.dma_start(cc_in[:], dt_sb[:])
            cc_out = drm.tile([NCORES * GP, K], F32, addr_space="Shared")
            nc.gpsimd.collective_compute(
                "AllGather",
                mybir.AluOpType.bypass,
                replica_groups=[list(range(NCORES))],
                ins=[cc_in.opt()],
                outs=[cc_out.opt()],
            )
            allg = sml.tile([GP, NCORES, K], F32)
            nc.sync.dma_start(
                allg[:], cc_out.rearrange("(r g) k -> g r k", r=NCORES)
            )

            dt_sum = sml.tile([GP, K], F32)
            nc.vector.tensor_add(dt_sum[:], allg[:, 0, :], allg[:, 1, :])
            for r in range(2, NCORES):
                nc.vector.tensor_add(dt_sum[:], dt_sum[:], allg[:, r, :])

            # negate so max_index finds the argmin; mask padded slots.
            nc.vector.tensor_scalar(
                dt_sum[:], dt_sum[:], -1.0, None, op0=mybir.AluOpType.mult
            )
            nc.vector.tensor_scalar_add(dt_sum[:], dt_sum[:], pen_t[:])
            ps_t = psp.tile([K, GP], F32)
            nc.tensor.transpose(ps_t[:], dt_sum[:], idn_t[:])
            ce_t = sml.tile([K, GP], F32)
            nc.vector.tensor_copy(ce_t[:], ps_t[:])

            mx = sml.tile([K, 8], F32)
            nc.vector.max(mx[:], ce_t[:])
            idx = sml.tile([K, 8], mybir.dt.uint32)
            nc.vector.max_index(idx[:], mx[:], ce_t[:])
            nc.sync.dma_start(out_d.ap(), idx[:, 0:1].bitcast(mybir.dt.int32))

    nc.compile()
    return nc


def _prepare_in_maps(segmentation, gt_instance, gt_plane_num, mode):
    seg = np.asarray(segmentation, dtype=np.float32)
    assert seg.shape == (N_FULL, K)
    seg = np.clip(np.rint(seg * 65536.0), 0.0, 65535.0).astype(np.uint16)
    gt = np.asarray(gt_instance)
    gmax = gt.shape[0]
    gpn = int(gt_plane_num)

    # (N, GP) bf16 mask matrix, padded columns zero.
    gpad = np.zeros((N_FULL, GP), dtype=np.float32)
    gpad[:, :gmax] = gt.reshape(gmax, -1).T
    gpad = gpad.astype(ml_dtypes.bfloat16)

    bias2 = np.empty((128, 2), dtype=np.float32)
    bias2[:, 0] = EPS
    bias2[:, 1] = 1.0 + EPS

    pen = np.zeros((GP, 1), dtype=np.float32)
    pen[min(gpn, GP):] = -1.0e30
    idn = np.eye(GP, dtype=np.float32)

    in_maps = []
    for c in range(NCORES):
        lo = c * NSHARD
        gt_core = (
            gpad[lo : lo + NSHARD]
            .reshape(NCHUNK, CHUNK, GP)
            .transpose(1, 0, 2)
            .reshape(CHUNK, NCHUNK * GP)
        )
        seg_core = (
            seg[lo : lo + NSHARD]
            .reshape(NCHUNK, CHUNK, K)
            .transpose(1, 0, 2)
            .reshape(CHUNK, NCHUNK * K)
        )
        m = {
            "seg": np.ascontiguousarray(seg_core),
            "gt": np.ascontiguousarray(gt_core),
            "bias2": bias2,
        }
        if mode == "device":
            m["pen"] = pen
            m["idn"] = idn
        in_maps.append(m)
    return in_maps


LAST_RESULTS = None


def run(inputs, trace=False, mode=None, **kwargs):
    global LAST_RESULTS
    mode = mode or MODE
    if mode not in _PROG:
        _PROG[mode] = _build_program(mode)
    in_maps = _prepare_in_maps(
        inputs["segmentation"], inputs["gt_instance"], inputs["gt_plane_num"], mode
    )
    res = run_bass_kernel_spmd(
        _PROG[mode], in_maps, core_ids=list(range(NCORES)), trace=trace, **kwargs
    )
    LAST_RESULTS = res
    if mode == "device":
        return np.asarray(res.results[0]["out"], dtype=np.int32)
    # gather/unshard: sum per-core partial (GP,2K) A|C matrices, form C - A,
    # mask padded instance slots, argmin over g (== argmin of the BCE).
    gpn = int(inputs["gt_plane_num"])
    d = np.sum([np.asarray(r["out"], np.float64) for r in res.results], axis=0)
    d[min(gpn, GP):, :] = np.inf
    return d.argmin(axis=0).astype(np.int32).reshape(K, 1)


def kernel(**inputs):
    return run(inputs)



# revision 2
# speedup vs baseline: 1.0011x; 1.0011x over previous
"""Trainium2 Bass kernel for nn_MatchSegmentation.

matching[k] = argmin_g ce[k,g], ce = mean_n BCE(segmentation[n,k], gt[g,n]).
Since B[k] = sum_n log(1-s+eps) is constant per k and -1/n is a negative
scale, argmin_g ce[k,:] == argmin_g (C-A)[k,:] with
  A = g @ log(s+eps)^T partials,  C = g @ log(1-s+eps)^T partials.

Sharding: pixels split 8 ways (8192/core). Per core:
  - seg is host-quantized to uint16 (u = round(s*65536); the <=2^-17
    quantization error is ~40x below the argmin safety margin) and
    host-swizzled so partition p holds pixels {c*128+p}: seg[p, c*K+k].
  - DMA (4 blocks, HWDGE sync queue) -> SBUF
  - ACT computes log(u*2^-16 + eps) and log(-u*2^-16 + 1+eps) per block
    into a concatenated (128, nch, 2K) bf16 tile (scalar engine Ln, the
    free input affine gives both logs from the same uint16 data)
  - PE accumulates psAC[g, 0:K] += gt_c^T @ log_s, psAC[g, K:2K] += ..log_1ms
    (one 256-wide bf16 matmul per 128-pixel chunk, fp32 PSUM)
  - epilogue: PSUM -> SBUF copy, DMA out the (22, 256) A|C partial sums.
Host: sum the 8 partials, D = C-A, mask padded g slots, argmin -> (K,1).

The engine-time budget per core: ACT 2 passes = 16384 cycles @1.2GHz =
13.7us (the bound), DMA 2.4MB @358GB/s = 6.8us, PE 16384 moving columns
@2.4GHz = 7us; DMA and PE hide under ACT.
"""

import numpy as np
import ml_dtypes
from contextlib import ExitStack

import concourse.bass as bass
import concourse.tile as tile
from concourse import bacc, mybir
from concourse.bass_utils import run_bass_kernel_spmd

F32 = mybir.dt.float32
BF16 = mybir.dt.bfloat16
U16 = mybir.dt.uint16

NCORES = 8
N_FULL = 65536          # h*w pixels
K = 128                 # segmentation channels
GMAX = 21               # gt instances provided
GP = 22                 # padded instance slots (col 21 always padding)
NSHARD = N_FULL // NCORES   # 8192 pixels per core
CHUNK = 128             # pixels per matmul (contraction = partition dim)
NCHUNK = NSHARD // CHUNK    # 64
EPS = 1e-6

# DMA blocks (chunks per seg dma_start) and ACT blocks (chunks per Ln
# instruction pair). ACT blocks must nest inside DMA blocks. Small first
# block -> early ACT start; small last block -> short matmul tail.
DMA_BLOCKS = [4, 12, 24, 24]
ACT_BLOCKS = [4, 12, 24, 20, 4]
assert sum(DMA_BLOCKS) == NCHUNK and sum(ACT_BLOCKS) == NCHUNK
N_WARM_MM = 12          # dummy matmuls to pull the PE HAM clock-gate open

_PROG = {}
MODE = "devlog"         # "devlog": logs on device; "hostlog": logs on host


def _build_program(mode):
    nc = bacc.Bacc(
        "TRN2",
        target_bir_lowering=False,
        debug=False,
        enable_asserts=False,
        num_devices=NCORES,
    )

    devlog = mode == "devlog"
    if devlog:
        seg_d = nc.dram_tensor("seg", [128, NCHUNK * K], U16, kind="ExternalInput")
        bias_d = nc.dram_tensor("bias2", [128, 2], F32, kind="ExternalInput")
        out_w = 2 * K
    else:
        seg_d = nc.dram_tensor("seg", [128, NCHUNK * K], BF16, kind="ExternalInput")
        out_w = K
    gt_d = nc.dram_tensor("gt", [128, NCHUNK * GP], BF16, kind="ExternalInput")
    out_d = nc.dram_tensor("out", [GP, out_w], F32, kind="ExternalOutput")

    with tile.TileContext(nc) as tc, ExitStack() as ctx:
        segp = ctx.enter_context(tc.tile_pool(name="segp", bufs=1))
        logp = ctx.enter_context(tc.tile_pool(name="logp", bufs=1))
        gtp = ctx.enter_context(tc.tile_pool(name="gtp", bufs=1))
        psp = ctx.enter_context(tc.tile_pool(name="psp", bufs=1, space="PSUM"))
        sml = ctx.enter_context(tc.tile_pool(name="sml", bufs=1))

        seg_ap = seg_d.ap()
        gt_ap = gt_d.ap()

        # --- t=0 prefetches and warmups, all on distinct queues ---
        # seg blocks stream on the sync HWDGE queue
        seg_tiles = []
        off = 0
        for b, nch in enumerate(DMA_BLOCKS):
            st = segp.tile([128, nch, K], seg_d.dtype, name=f"seg{b}", tag=f"seg{b}")
            nc.sync.dma_start(
                st[:],
                seg_ap[:, off * K : (off + nch) * K].rearrange(
                    "p (c k) -> p c k", c=nch
                ),
            )
            seg_tiles.append((off, st))
            off += nch

        # gt + bias prefetch on the gpsimd SWDGE queue (kept off sync/scalar)
        gt_t = gtp.tile([128, NCHUNK, GP], BF16)
        nc.gpsimd.dma_start(
            gt_t[:], gt_ap.rearrange("p (c j) -> p c j", c=NCHUNK)
        )
        if devlog:
            bias_t = sml.tile([128, 2], F32)
            nc.gpsimd.dma_start(bias_t[:], bias_d.ap())

            # Warm the ACT Ln table at t=0 (1.3us load hides under DMA).
            dummy = sml.tile([1, 8], F32)
            nc.vector.memset(dummy[:], 1.0)
            nc.scalar.activation(dummy[:], dummy[:], mybir.ActivationFunctionType.Ln)

        # PE HAM warmup: back-to-back dummy matmuls into a scratch PSUM bank
        # while the first seg block is still in flight.
        if N_WARM_MM:
            wl = sml.tile([128, GP], BF16)
            wr = sml.tile([128, K], BF16)
            nc.vector.memset(wl[:], 0.0)
            nc.vector.memset(wr[:], 0.0)
            ps_w = psp.tile([GP, K], F32, name="ps_warm", tag="ps_warm")
            for i in range(N_WARM_MM):
                nc.tensor.matmul(ps_w[:], lhsT=wl[:], rhs=wr[:], start=True, stop=True)

        # --- main pipeline: ACT (2 Ln passes per block) + PE accumulate ---
        psAC = psp.tile([GP, out_w], F32, name="psAC", tag="psAC")

        def seg_slice(off, nch):
            """View of chunks [off, off+nch) inside its DMA-block tile."""
            for boff, st in seg_tiles:
                if boff <= off and off + nch <= boff + st.shape[1]:
                    return st[:, off - boff : off - boff + nch, :]
            raise AssertionError("ACT block not nested in a DMA block")

        gc = 0
        for a, nch in enumerate(ACT_BLOCKS):
            if devlog:
                lt = logp.tile([128, nch, 2 * K], BF16, name=f"log{a}", tag=f"log{a}")
                ss = seg_slice(gc, nch)
                nc.scalar.activation(
                    lt[:, :, 0:K], ss,
                    mybir.ActivationFunctionType.Ln,
                    bias=bias_t[:, 0:1], scale=1.0 / 65536.0,
                )
                nc.scalar.activation(
                    lt[:, :, K : 2 * K], ss,
                    mybir.ActivationFunctionType.Ln,
                    bias=bias_t[:, 1:2], scale=-1.0 / 65536.0,
                )
            else:
                lt = seg_slice(gc, nch)

            for c in range(nch):
                nc.tensor.matmul(
                    psAC[:],
                    lhsT=gt_t[:, gc + c, :],
                    rhs=lt[:, c, :],
                    start=(gc + c == 0),
                    stop=(gc + c == NCHUNK - 1),
                )
            gc += nch

        # --- epilogue: PSUM -> SBUF -> HBM; host reduces across cores ---
        ac_sb = sml.tile([GP, out_w], F32)
        nc.vector.tensor_copy(ac_sb[:], psAC[:])
        nc.sync.dma_start(out_d.ap(), ac_sb[:])

    nc.compile()
    return nc


def _prepare_in_maps(segmentation, gt_instance, mode):
    seg = np.asarray(segmentation, dtype=np.float32)
    assert seg.shape == (N_FULL, K)
    if mode == "devlog":
        seg = np.clip(np.rint(seg * 65536.0), 0.0, 65535.0).astype(np.uint16)
    else:
        d = np.log(1.0 - seg + EPS) - np.log(seg + EPS)
        seg = d.astype(ml_dtypes.bfloat16)
    gt = np.asarray(gt_instance)
    gmax = gt.shape[0]

    gpad = np.zeros((N_FULL, GP), dtype=np.float32)
    gpad[:, :gmax] = gt.reshape(gmax, -1).T
    gpad = gpad.astype(ml_dtypes.bfloat16)

    bias2 = np.empty((128, 2), dtype=np.float32)
    bias2[:, 0] = EPS
    bias2[:, 1] = 1.0 + EPS

    in_maps = []
    for c in range(NCORES):
        lo = c * NSHARD
        gt_core = (
            gpad[lo : lo + NSHARD]
            .reshape(NCHUNK, CHUNK, GP)
            .transpose(1, 0, 2)
            .reshape(CHUNK, NCHUNK * GP)
        )
        seg_core = (
            seg[lo : lo + NSHARD]
            .reshape(NCHUNK, CHUNK, K)
            .transpose(1, 0, 2)
            .reshape(CHUNK, NCHUNK * K)
        )
        m = {
            "seg": np.ascontiguousarray(seg_core),
            "gt": np.ascontiguousarray(gt_core),
        }
        if mode == "devlog":
            m["bias2"] = bias2
        in_maps.append(m)
    return in_maps


LAST_RESULTS = None


def run(inputs, trace=False, mode=None, **kwargs):
    global LAST_RESULTS
    mode = mode or MODE
    if mode not in _PROG:
        _PROG[mode] = _build_program(mode)
    in_maps = _prepare_in_maps(inputs["segmentation"], inputs["gt_instance"], mode)
    res = run_bass_kernel_spmd(
        _PROG[mode], in_maps, core_ids=list(range(NCORES)), trace=trace, **kwargs
    )
    LAST_RESULTS = res
    gpn = int(inputs["gt_plane_num"])
    acc = np.sum([np.asarray(r["out"], np.float64) for r in res.results], axis=0)
    if mode == "devlog":
        d = acc[:, K : 2 * K] - acc[:, 0:K]   # C - A, (GP, K)
    else:
        d = acc                               # already sum g*(log1ms-logs)
    d[min(gpn, GP):, :] = np.inf
    return d.argmin(axis=0).astype(np.int32).reshape(K, 1)


def kernel(**inputs):
    return run(inputs)
